# revision 3
# baseline (speedup 1.0000x reference)
"""Trainium2 Bass kernel v6: normal orientation, per-tick uncoupled
stream interleave + host-precomputed x-gate contributions.

vs kernel5: the K=7 x-matmuls (measured ~190ns each due to poor
column efficiency) are replaced by identity-matmul injection of
host-precomputed xg tiles (N=128 @ ~85ns), which also carries both
layers' biases. The 4 injections are emitted before the 4 weight
matmuls each tick, so only the weight matmuls sit on the h-recurrence
chain. xg is staged into SBUF in XBLK-tick blocks by DMA.

Layout recap (see kernel2.py): gate-major [L1;L2] partition stacking,
all-tanh trick, h'=2h, D=2c state.
  psQ[:, X*BS:(X+1)*BS] = gate-X pre-activations, partitions =
    (64 L1 units; 64 L2 units), X in (i,f,g,o).
  tick k: L1 computes h1(k), L2 computes h2(k-1).
"""

import numpy as np

H = 64
I = 6
O = 6
NCORES = 8


def _build(nc, tc, BL, BS, T, XBLK):
    import concourse.bass as bass
    from concourse import mybir

    f32 = mybir.dt.float32
    bf16 = mybir.dt.bfloat16
    AF = mybir.ActivationFunctionType
    OP = mybir.AluOpType
    NS = BL // BS

    # xg: host-precomputed x/bias gate contributions [T, 4, 128, BL]
    xg_d = nc.dram_tensor("xg", [T, 4, 128, BL], bf16, kind="ExternalInput")
    wbig_d = nc.dram_tensor("wbig", [128, 512], bf16, kind="ExternalInput")
    wl_d = nc.dram_tensor("wl", [128, O], bf16, kind="ExternalInput")
    id_d = nc.dram_tensor("ident", [128, 128], bf16, kind="ExternalInput")
    y_d = nc.dram_tensor("y", [O, BL], f32, kind="ExternalOutput")

    import contextlib
    ctx = contextlib.ExitStack()
    wp = ctx.enter_context(tc.tile_pool(name="w", bufs=1))
    mp = ctx.enter_context(tc.tile_pool(name="m2", bufs=2))
    dp = ctx.enter_context(tc.tile_pool(name="dst", bufs=2))
    xp = ctx.enter_context(tc.tile_pool(name="xs", bufs=2))
    t4p = ctx.enter_context(tc.tile_pool(name="t4", bufs=2))
    tcp = ctx.enter_context(tc.tile_pool(name="tc", bufs=2))
    uvp = ctx.enter_context(tc.tile_pool(name="uv", bufs=2))
    sp = ctx.enter_context(tc.tile_pool(name="s", bufs=2))
    pp = ctx.enter_context(tc.tile_pool(name="ps", bufs=1, space="PSUM"))

    wbig = wp.tile([128, 512], bf16, tag="wbig")
    nc.sync.dma_start(out=wbig, in_=wbig_d[:, :])
    wl = wp.tile([128, O], bf16, tag="wl")
    nc.sync.dma_start(out=wl, in_=wl_d[:, :])
    ident = wp.tile([128, 128], bf16, tag="ident")
    nc.sync.dma_start(out=ident, in_=id_d[:, :])

    m2 = [[mp.tile([128, BS], bf16, tag=f"m2{s}", name=f"m2_{s}_{j}")
           for j in range(2)] for s in range(NS)]
    dst = [[dp.tile([128, BS], f32, tag=f"d{s}", name=f"d_{s}_{j}")
            for j in range(2)] for s in range(NS)]
    # one tile spanning 4 PSUM banks; gate X at a fixed one-bank offset
    psq = [pp.tile([128, 2048], f32, tag=f"pq{s}", name=f"pq_{s}")
           for s in range(NS)]
    # xg staging: [128, XBLK*4*BS] per stream, double buffered
    xst = [[xp.tile([128, XBLK * 4 * BS], bf16, tag=f"xs{s}",
                    name=f"xs_{s}_{j}") for j in range(2)] for s in range(NS)]

    def xg_dma(s, blk):
        nc.sync.dma_start(
            out=xst[s][blk % 2][:, :].rearrange(
                "p (t x b) -> p t x b", t=XBLK, x=4),
            in_=xg_d[blk * XBLK:(blk + 1) * XBLK, :, :,
                     s * BS:(s + 1) * BS].rearrange("t x p b -> p t x b"))

    for s in range(NS):
        for t_ in m2[s]:
            nc.vector.memset(t_[:, :], 0.0)
        nc.vector.memset(dst[s][0][:, :], 0.0)
        xg_dma(s, 0)

    Bi = slice(0, BS)
    Bf = slice(BS, 2 * BS)
    Bg = slice(2 * BS, 3 * BS)
    Bo = slice(3 * BS, 4 * BS)

    for k in range(T + 1):
        if k % XBLK == 0 and k + XBLK < T:
            for s in range(NS):
                xg_dma(s, k // XBLK + 1)

        kx = min(k, T - 1)  # at k==T reuse xg(T-1); L1 result is discarded
        xb = (kx // XBLK) % 2
        xo = (kx % XBLK) * 4 * BS

        for s in range(NS):
            mv = m2[s][k % 2]
            # gate X lives in its own PSUM bank at cols X*4*BS; groups in
            # different banks may interleave, so all injections go first
            for X in range(4):
                nc.tensor.matmul(
                    psq[s][:, X * 512:X * 512 + BS], ident,
                    xst[s][xb][:, xo + X * BS:xo + (X + 1) * BS],
                    start=True, stop=False)
            for X in range(4):
                nc.tensor.matmul(psq[s][:, X * 512:X * 512 + BS],
                                 wbig[:, X * 128:(X + 1) * 128], mv,
                                 start=False, stop=True)

            T4 = t4p.tile([128, 4 * BS], bf16, tag=f"t4{s}", name=f"T4{s}")
            nc.scalar.activation(
                T4[:, :].rearrange("p (x c) -> p x c", x=4),
                psq[s][:, :].rearrange("p (x c) -> p x c", x=4)[:, :, 0:BS],
                AF.Tanh)

            u = uvp.tile([128, BS], bf16, tag=f"u{s}", name=f"u{s}")
            v = uvp.tile([128, BS], f32, tag=f"v{s}", name=f"v{s}")
            dn = dst[s][(k + 1) % 2]
            nc.vector.scalar_tensor_tensor(
                out=v[:, :], in0=T4[:, Bf], scalar=1.0, in1=dst[s][k % 2][:, :],
                op0=OP.add, op1=OP.mult)
            nc.vector.scalar_tensor_tensor(
                out=u[:, :], in0=T4[:, Bi], scalar=1.0, in1=T4[:, Bg],
                op0=OP.add, op1=OP.mult)
            nc.vector.scalar_tensor_tensor(
                out=dn[:, :], in0=v[:, :], scalar=0.5, in1=u[:, :],
                op0=OP.mult, op1=OP.add)
            TC = tcp.tile([128, BS], bf16, tag=f"tc{s}", name=f"TC{s}")
            nc.scalar.activation(TC[:, :], dn[:, :], AF.Tanh, scale=0.5)
            nc.vector.scalar_tensor_tensor(
                out=m2[s][(k + 1) % 2][:, :], in0=T4[:, Bo], scalar=1.0,
                in1=TC[:, :], op0=OP.add, op1=OP.mult)

            if k == 0:
                nc.vector.memset(m2[s][1][64:128, :], 0.0)
                nc.vector.memset(dst[s][1][64:128, :], 0.0)

    for s in range(NS):
        psF = psq[s][0:O, BS:2 * BS]
        nc.tensor.matmul(psF, wl, m2[s][(T + 1) % 2],
                         start=True, stop=True)
        oF = sp.tile([O, BS], f32, tag=f"oF{s}", name=f"oF{s}")
        nc.vector.tensor_copy(oF[:, :], psF)
        nc.sync.dma_start(out=y_d[:, s * BS:(s + 1) * BS], in_=oF)

    ctx.close()


def build_nc(BL=256, BS=128, T=512, XBLK=8):
    import concourse.bacc as bacc
    import concourse.tile as tile

    nc = bacc.Bacc(None, target_bir_lowering=False)
    with tile.TileContext(nc) as tc:
        _build(nc, tc, BL, BS, T, XBLK)
    nc.compile()
    return nc


def prep_weights(Wih0, Whh0, bih0, bhh0, Wih1, Whh1, bih1, bhh1, Wlin, blin):
    import ml_dtypes
    bf = ml_dtypes.bfloat16
    f = np.float32

    wbig = np.zeros((128, 512), f)
    for X in range(4):
        sX = 1.0 if X == 2 else 0.5
        r = slice(X * 64, (X + 1) * 64)
        c = slice(X * 128, X * 128 + 64)
        c2 = slice(X * 128 + 64, X * 128 + 128)
        wbig[0:64, c] = (sX * 0.5) * Whh0[r].T
        wbig[0:64, c2] = (sX * 0.5) * Wih1[r].T
        wbig[64:128, c2] = (sX * 0.5) * Whh1[r].T

    wl = np.zeros((128, O), f)
    wl[64:128, :] = 0.5 * Wlin.T
    return {"wbig": wbig.astype(bf), "wl": wl.astype(bf)}


def prep_xg(x, Wih0, bih0, bhh0, bih1, bhh1):
    """xg[t, X, p, b]: p 0:64 = sX*(Wih0_X @ x(t,b) + b0_X);
    p 64:128 = sX * b1_X (L2 bias, broadcast over t)."""
    import ml_dtypes
    f = np.float32
    B, T, _ = x.shape
    b0 = (bih0 + bhh0).astype(f)
    b1 = (bih1 + bhh1).astype(f)
    # [B*T, 6] @ [6, 256] -> [B, T, 256]
    g1 = (x.reshape(-1, I) @ Wih0.T.astype(f)).reshape(B, T, 4 * H) + b0
    xg = np.empty((T, 4, 128, B), dtype=ml_dtypes.bfloat16)
    for X in range(4):
        sX = 1.0 if X == 2 else 0.5
        xg[:, X, 0:64, :] = (sX * g1[:, :, X * 64:(X + 1) * 64]).transpose(1, 2, 0)
        xg[:, X, 64:128, :] = (sX * b1[X * 64:(X + 1) * 64])[None, :, None]
    return xg


_NC_CACHE = {}


def kernel(x, Wih0, Whh0, bih0, bhh0, Wih1, Whh1, bih1, bhh1, Wlin, blin,
           _trace=False):
    from concourse.bass_utils import run_bass_kernel_spmd

    x = np.asarray(x, dtype=np.float32)
    B, T, _ = x.shape
    BL = B // NCORES
    key = (BL, T)
    if key not in _NC_CACHE:
        _NC_CACHE[key] = build_nc(BL=BL, BS=BL // 2, T=T)
    nc = _NC_CACHE[key]

    w = prep_weights(np.asarray(Wih0), np.asarray(Whh0), np.asarray(bih0),
                     np.asarray(bhh0), np.asarray(Wih1), np.asarray(Whh1),
                     np.asarray(bih1), np.asarray(bhh1), np.asarray(Wlin),
                     np.asarray(blin))
    xg = prep_xg(x, np.asarray(Wih0), np.asarray(bih0), np.asarray(bhh0),
                 np.asarray(bih1), np.asarray(bhh1))

    import ml_dtypes
    ident = np.eye(128, dtype=ml_dtypes.bfloat16)
    in_maps = []
    for c in range(NCORES):
        m = {"xg": np.ascontiguousarray(xg[:, :, :, c * BL:(c + 1) * BL]),
             "ident": ident}
        m.update(w)
        in_maps.append(m)

    res = run_bass_kernel_spmd(nc, in_maps, core_ids=list(range(NCORES)),
                               trace=_trace)
    yT = np.concatenate([r["y"] for r in res.results], axis=1)
    out = yT.T.astype(np.float32) + np.asarray(blin, dtype=np.float32)[None, :]
    if _trace:
        kernel._last_result = res
    return out
